# revision 27
# baseline (speedup 1.0000x reference)
"""Multi-head attention (B=2, S=2048, D=1024, H=16) on 8 TRN2 NeuronCores.

Sharding: core c -> (batch b = c//4, head-group g = c%4). Each core computes
the attention output restricted to its batch and its 4 heads (a 256-wide
slice of the model dim), including the row-parallel output projection
partial product. Host sums the 4 partials per batch and adds bo.

Device-side layouts (everything transposed so no on-device transposes are
needed):
  xq/xk/xv  bf16 [1025, 2048]  = x[b].T with a trailing ones row (bias trick)
  wq/wk/wv  bf16 [1025, 256]   = W[g-slice, :].T with trailing bias row
  wo        bf16 [256, 1024]   = Wo[:, g-slice].T
  outT      f32  [1024, 2048]  = (Wo_g @ ctxn_g^T) partial, host transposes

Pipeline per core (PE never idles; DMA ordered to match consumption):
  K^T dch0,dch1 = Wk xk^T    (PE, psum accum; xk DMA'd first)
  Q^T dch0,dch1 = Wq xq^T
  per head pair (dch): per hf (q half):
    scores^T[k,q] = K_h^T-stationary @ Q_h^T   (64-row-tiled PE)
    attn = exp(scores/8)                        (ACT, psum->sbuf bf16)
    ctx_aug^T[d+1,q] += V_aug^T-stationary @ attn (psum accum, LAG behind)
    V projection grains interleave into head 0 (natural [s,d] layout)
    finished ctxn regions' out-projection grains interleave 1/chunk
  normalize: cp copy, reciprocal_approx_fast, gpsimd bcast, DVE mult
  tail (h3,hf1): ctx split into two 512-wide psums so normalize/outproj
  of the first half overlap the second half's PV.
"""

import numpy as np
import ml_dtypes

from concourse import bacc, tile, mybir
from concourse.bass_utils import run_bass_kernel_spmd

BF16 = mybir.dt.bfloat16
F32 = mybir.dt.float32
FP16 = mybir.dt.float16

S = 2048      # sequence length
D = 1024      # model dim
DG = 256      # per-core head-group width (4 heads x 64)
DK = 64       # head dim
NH = 4        # heads per core
MT = 8        # model-dim contraction tiles (1024 / 128)
KC = 16       # k chunks of 128
LAG = 4       # PV trails scores by LAG chunks
N_CORES = 8


def _emit_init(nc, pools, dram):
    """Iteration-invariant prologue: weight DMAs, ones row, bias row."""
    persist, xp, wp, wop, attnp, zp, outp, ps, ctxps, smallps = pools
    xq, xk, xv, wq, wk, wv, bT, wo, outT0, outT1 = dram

    ones = persist.tile([1, S], BF16, tag="ones", name="ones")
    nc.vector.memset(ones[:], 1.0)
    bt = persist.tile([1, 3 * DG], BF16, tag="bt", name="bt")
    nc.sync.dma_start(bt[:], bT[:])
    onesw = persist.tile([1, NH], BF16, tag="onesw", name="onesw")
    nc.vector.memset(onesw[:], 1.0)

    wts = {}
    for key, wdr in (("k", wk), ("q", wq), ("v", wv)):
        wt = []
        for m in range(MT):
            t = wp.tile([128, DG], BF16, tag="w", name="w")
            nc.sync.dma_start(t[:], wdr[m * 128:(m + 1) * 128, :])
            wt.append(t)
        wts[key] = wt

    wot = []
    for dch in range(2):
        t = wop.tile([128, D], BF16, tag="wo", name="wo")
        nc.sync.dma_start(t[:], wo[dch * 128:(dch + 1) * 128, :])
        wot.append(t)
    return ones, bt, onesw, wts, wot


def _emit(nc, pools, dram, init):
    persist, xp, wp, wop, attnp, zp, outp, ps, ctxps, smallps = pools
    xq, xk, xv, wq, wk, wv, bT, wo, outT0, outT1 = dram
    ones, bt, onesw, wts, wot = init
    HS = S // 2  # 1024-wide half grains

    # per-iteration persistent tiles (double-buffered across iterations)
    qt = [persist.tile([128, S], BF16, tag=f"qt{i}", name=f"qt{i}", bufs=2)
          for i in range(2)]
    kt = [persist.tile([128, S], BF16, tag=f"kt{i}", name=f"kt{i}", bufs=2)
          for i in range(2)]
    ctxn = [persist.tile([128, S], BF16, tag=f"ctxn{i}", name=f"ctxn{i}",
                         bufs=2) for i in range(2)]
    vaug = persist.tile([128, KC, NH, DK + 1], BF16, tag="vaug", name="vaug",
                        bufs=2)

    # input DMAs go on the gpsimd SW-DGE queue: the Sync queue carries the
    # output DMAs, and at the iteration seam the next body's input issues
    # would otherwise serialize behind ~32 queued output issues (~10us)
    xts = {}
    for key, xdr in (("k", xk), ("q", xq), ("v", xv)):
        xt = [xp.tile([128, S], BF16, tag="x", name=f"x{key}") for _ in range(MT)]
        for hf in range(2):
            for m in range(MT):
                nc.gpsimd.dma_start(xt[m][:, hf * HS:(hf + 1) * HS],
                                    xdr[m * 128:(m + 1) * 128, hf * HS:(hf + 1) * HS])
        xts[key] = xt

    # ---------------- K^T then Q^T projections (both dch up front) ---------
    # 512-wide psum grains, sharing the "ps" tag with the attention scores
    # psums: under cross-iteration overlap the next iteration's projections
    # run concurrently with this iteration's attention, and PSUM only has 8
    # banks (2 here + 4 ctx + 2 small).
    for bofs, (key, outsb) in enumerate((("k", kt), ("q", qt))):
        wt, xt = wts[key], xts[key]
        for dch in range(2):
            for hf in range(2):
                for qc in range(2):
                    psum = ps.tile([128, 512], F32, tag="ps", name="ps")
                    for m in range(MT):
                        nc.tensor.matmul(
                            psum[:],
                            wt[m][:, dch * 128:(dch + 1) * 128],
                            xt[m][:, hf * HS + qc * 512:hf * HS + (qc + 1) * 512],
                            start=(m == 0), stop=False)
                    bo_c = (1 - bofs) * DG + dch * 128  # bt: [bq, bk, bv]
                    nc.tensor.matmul(
                        psum[:],
                        bt[:, bo_c:bo_c + 128],
                        ones[:, hf * HS + qc * 512:hf * HS + (qc + 1) * 512],
                        start=False, stop=True)
                    nc.vector.tensor_copy(
                        outsb[dch][:, hf * HS + qc * 512:hf * HS + (qc + 1) * 512],
                        psum[:])

    # ---------------- V projection grains (interleaved into head 0) --------
    def vproj_grain(sc):
        xt = xts["v"]
        vps = smallps.tile([128, NH * (DK + 1)], F32, tag="sm", name="vps")
        for m in range(MT):
            nc.tensor.matmul(
                vps[:, 0:NH * DK],
                xt[m][:, sc * 128:(sc + 1) * 128],
                wts["v"][m][:],
                start=(m == 0), stop=False)
        nc.tensor.matmul(
            vps[:, 0:NH * DK],
            ones[:, sc * 128:(sc + 1) * 128],
            bt[:, 2 * DG:3 * DG],
            start=False, stop=True)
        # the softmax-denominator ones column, via a K=1 matmul (writing it
        # here instead of a per-iteration memset keeps vaug fully produced
        # by the loop body, so double-buffering works across iterations)
        nc.tensor.matmul(
            vps[:, NH * DK:NH * (DK + 1)],
            ones[:, sc * 128:(sc + 1) * 128],
            onesw[:],
            start=True, stop=True, skip_group_check=True)
        nc.vector.tensor_copy(vaug[:, sc, :, 0:DK],
                              vps[:, 0:NH * DK].rearrange(
                                  "p (h d) -> p h d", h=NH))
        nc.vector.tensor_copy(vaug[:, sc, :, DK:DK + 1],
                              vps[:, NH * DK:NH * (DK + 1)].rearrange(
                                  "p (h d) -> p h d", h=NH))

    # ---------------- out-projection grains --------------------------------
    def outproj_grain(dch, oc, qp, outT):
        """A [128, 1024] out-projection pair: two 512-wide psum grains,
        evicted into one SBUF tile, shipped with a single DMA."""
        osb = outp.tile([128, HS], BF16, tag="out", name="out")
        for j in range(2):
            ops = smallps.tile([128, 512], F32, tag="sm", name="ops")
            nc.tensor.matmul(
                ops[:], wot[dch][:, oc * 128:(oc + 1) * 128],
                ctxn[dch][:, qp * HS + j * 512:qp * HS + (j + 1) * 512],
                start=True, stop=True)
            nc.vector.tensor_copy(osb[:, j * 512:(j + 1) * 512], ops[:])
        nc.sync.dma_start(
            outT[oc * 128:(oc + 1) * 128, qp * HS:(qp + 1) * HS], osb[:])

    def outproj_half(dch, oc, qp, j, outT):
        """512-wide out-projection grain (tail variant)."""
        osb = outp.tile([128, HS], BF16, tag="out", name="outh")
        ops = smallps.tile([128, 512], F32, tag="sm", name="ops")
        nc.tensor.matmul(
            ops[:], wot[dch][:, oc * 128:(oc + 1) * 128],
            ctxn[dch][:, qp * HS + j * 512:qp * HS + (j + 1) * 512],
            start=True, stop=True)
        nc.vector.tensor_copy(osb[:, 0:512], ops[:])
        nc.sync.dma_start(
            outT[oc * 128:(oc + 1) * 128,
                 qp * HS + j * 512:qp * HS + (j + 1) * 512], osb[:, 0:512])

    def normalize(ctx_ap, dch, po, col0, width, via_cp=True, sfx=""):
        """ctxn[dch][po:po+DK, col0:col0+width] = ctx[0:DK] / ctx[DK].

        The [1, width] reciprocal is hostile to the DVE (single partition,
        ~6 cycles/elem -> 6.5us that clogs the in-order DVE FIFO), so the Z
        row takes a DMA round-trip through a [128, width/128] staging tile
        where the reciprocal runs across all partitions in ~0.1us.
        """
        if via_cp:
            cp = zp.tile([DK + 1, width], F32, tag="cp" + sfx, name="cp")
            nc.vector.tensor_copy(cp[:], ctx_ap)
            src = cp
            zrow = cp[DK:DK + 1, :]
        else:
            src = ctx_ap
            zrow_t = zp.tile([1, width], F32, tag="zrow" + sfx, name="zrow")
            nc.vector.tensor_copy(zrow_t[:], ctx_ap[DK:DK + 1, :])
            zrow = zrow_t[:]
        w = width // 128
        zst = zp.tile([128, w], F32, tag="zst" + sfx, name="zst")
        nc.gpsimd.dma_start(zst[:], zrow)
        rst = zp.tile([128, w], F32, tag="rst" + sfx, name="rst")
        nc.vector.reciprocal(rst[:], zst[:])
        zr = zp.tile([1, width], F32, tag="zr" + sfx, name="zr")
        nc.gpsimd.dma_start(zr[:], rst[:])
        bc = zp.tile([DK, width], F32, tag="bc" + sfx, name="bc")
        nc.gpsimd.partition_broadcast(bc[:], zr[:])
        nc.vector.tensor_mul(ctxn[dch][po:po + DK, col0:col0 + width],
                             src[0:DK, :], bc[:])

    # ---------------- attention, fused head pairs ---------------------------
    # Heads of a pair (dch) live on PE row-tiles T0 (partitions 0-63) and T8
    # (64-127); interleaving their scores matmuls makes the two 64-row tiles
    # stream concurrently (~2x scores throughput). Scores psums are evicted
    # to fp16 SBUF staging by the DVE immediately (freeing PSUM), and exp
    # runs on 2048-wide staged tiles, amortizing the ACT engine's 352-cycle
    # per-instruction overhead.
    from collections import deque
    pending = deque()
    for p in range(2):
        heads = ((2 * p, 0), (2 * p + 1, 64))
        for hf in range(2):
            last = (p == 1 and hf == 1)
            ctxs = [ctxps.tile([DK + 1, HS], F32, tag=f"ctx{t}",
                               name=f"ctx{t}") for t in range(2)]
            stage = [None, None]
            atts = {}
            for cc in range(KC + LAG):
                if cc < KC:
                    c = cc
                    if p == 0 and hf == 0:
                        vproj_grain(c)
                    if c % 2 == 0:
                        stage[0] = attnp.tile([128, S], FP16, tag="stg0",
                                              name="stg0")
                        stage[1] = attnp.tile([128, S], FP16, tag="stg1",
                                              name="stg1")
                    for qc in range(2):
                        for t, (hh, po) in enumerate(heads):
                            sp = ps.tile([128, 512], F32, tag="ps", name="ps")
                            nc.tensor.matmul(
                                sp[:],
                                kt[p][po:po + DK, c * 128:(c + 1) * 128],
                                qt[p][po:po + DK,
                                      hf * HS + qc * 512:hf * HS + (qc + 1) * 512],
                                start=True, stop=True)
                            nc.vector.tensor_copy(
                                stage[t][:, (c % 2) * HS + qc * 512:
                                         (c % 2) * HS + (qc + 1) * 512],
                                sp[:])
                    if c % 2 == 1:
                        for t in range(2):
                            att = attnp.tile([128, S], BF16, tag=f"att{t}",
                                             name=f"att{t}")
                            nc.scalar.activation(
                                att[:], stage[t][:],
                                mybir.ActivationFunctionType.Exp, scale=0.125)
                            atts[(t, c // 2)] = att
                if cc >= LAG:
                    c = cc - LAG
                    for qc in range(2):
                        for t, (hh, po) in enumerate(heads):
                            att = atts[(t, c // 2)]
                            nc.tensor.matmul(
                                ctxs[t][:, qc * 512:(qc + 1) * 512],
                                vaug[:, c, hh, :],
                                att[:, (c % 2) * HS + qc * 512:
                                    (c % 2) * HS + (qc + 1) * 512],
                                start=(c == 0), stop=(c == KC - 1))
                    if c % 2 == 1:
                        atts.pop((0, c // 2))
                        atts.pop((1, c // 2))
                if pending and cc >= 4 and cc % 2 == 0:
                    outproj_grain(*pending.popleft())
            for t, (hh, po) in enumerate(heads):
                normalize(ctxs[t][:], p, po, hf * HS, HS, via_cp=False,
                          sfx="B" if t else "")
            pending.extend((p, oc, hf, outT0 if p == 0 else outT1)
                           for oc in range(8))
    for g in pending:
        outproj_grain(*g)


def build_nc(reps=1):
    nc = bacc.Bacc("TRN2", target_bir_lowering=False)
    dram = (
        nc.dram_tensor("xq", [D, S], BF16, kind="ExternalInput"),
        nc.dram_tensor("xk", [D, S], BF16, kind="ExternalInput"),
        nc.dram_tensor("xv", [D, S], BF16, kind="ExternalInput"),
        nc.dram_tensor("wq", [D, DG], BF16, kind="ExternalInput"),
        nc.dram_tensor("wk", [D, DG], BF16, kind="ExternalInput"),
        nc.dram_tensor("wv", [D, DG], BF16, kind="ExternalInput"),
        nc.dram_tensor("bT", [1, 3 * DG], BF16, kind="ExternalInput"),
        nc.dram_tensor("wo", [DG, D], BF16, kind="ExternalInput"),
        nc.dram_tensor("outT0", [D, S], BF16, kind="ExternalOutput"),
        nc.dram_tensor("outT1", [D, S], BF16, kind="ExternalOutput"),
    )

    with tile.TileContext(nc) as tc:
        with (
            tc.tile_pool(name="persist", bufs=1) as persist,
            tc.tile_pool(name="xp", bufs=14) as xp,
            tc.tile_pool(name="wp", bufs=26) as wp,
            tc.tile_pool(name="wop", bufs=2) as wop,
            tc.tile_pool(name="attnp", bufs=2) as attnp,
            tc.tile_pool(name="zp", bufs=1) as zp,
            tc.tile_pool(name="outp", bufs=4) as outp,
            tc.tile_pool(name="ps", bufs=2, space="PSUM") as ps,
            tc.tile_pool(name="ctxps", bufs=1, space="PSUM") as ctxps,
            tc.tile_pool(name="smallps", bufs=2, space="PSUM") as smallps,
        ):
            pools = (persist, xp, wp, wop, attnp, zp, outp, ps, ctxps, smallps)
            init = _emit_init(nc, pools, dram)
            if reps == 1:
                _emit(nc, pools, dram, init)
            elif reps % 2 == 0:
                # two bodies per hardware-loop iteration: the loop-boundary
                # pipeline bubble (~11us) is paid once per two iterations
                with tc.For_i(0, reps // 2, 1):
                    _emit(nc, pools, dram, init)
                    _emit(nc, pools, dram, init)
            else:
                with tc.For_i(0, reps, 1):
                    _emit(nc, pools, dram, init)
    nc.compile()
    return nc


def make_in_maps(query, key, value, Wq, bq, Wk, bk, Wv, bv, Wo, bo):
    bf = ml_dtypes.bfloat16
    query, key, value = (np.asarray(a, np.float32) for a in (query, key, value))
    Wq, bq, Wk, bk, Wv, bv, Wo, bo = (
        np.asarray(a, np.float32) for a in (Wq, bq, Wk, bk, Wv, bv, Wo, bo))
    in_maps = []
    for c in range(N_CORES):
        b, g = divmod(c, 4)
        sl = slice(g * DG, (g + 1) * DG)

        def xa(x):
            return np.ascontiguousarray(x[b].T).astype(bf)

        def wa(W):
            return np.ascontiguousarray(W[sl, :].T).astype(bf)

        in_maps.append({
            "xq": xa(query), "xk": xa(key), "xv": xa(value),
            "wq": wa(Wq), "wk": wa(Wk), "wv": wa(Wv),
            "bT": np.concatenate([bq[sl], bk[sl], bv[sl]])[None, :].astype(bf),
            "wo": np.ascontiguousarray(Wo[:, sl].T).astype(bf),
        })
    return in_maps


_NC_CACHE = {}


def kernel(query, key, value, Wq, bq, Wk, bk, Wv, bv, Wo, bo):
    in_maps = make_in_maps(query, key, value, Wq, bq, Wk, bk, Wv, bv, Wo, bo)
    if 1 not in _NC_CACHE:
        _NC_CACHE[1] = build_nc(1)
    nc = _NC_CACHE[1]
    res = run_bass_kernel_spmd(nc, in_maps, core_ids=list(range(N_CORES)))
    out = np.zeros((2, S, D), np.float32)
    for c in range(N_CORES):
        b = c // 4
        out[b] += np.asarray(res.results[c]["outT0"], np.float32).T
        out[b] += np.asarray(res.results[c]["outT1"], np.float32).T
    out += np.asarray(bo, np.float32)[None, None, :]
    return out


# revision 28
# speedup vs baseline: 1.1307x; 1.1307x over previous
"""Multi-head attention (B=2, S=2048, D=1024, H=16) on 8 TRN2 NeuronCores.

Sharding: core c -> (batch b = c//4, head-group g = c%4). Each core computes
the attention output restricted to its batch and its 4 heads (a 256-wide
slice of the model dim), including the row-parallel output projection
partial product. Host sums the 4 partials per batch and adds bo.

Device-side layouts (everything transposed so no on-device transposes are
needed):
  xq/xk/xv  bf16 [1025, 2048]  = x[b].T with a trailing ones row (bias trick)
  wq/wk/wv  bf16 [1025, 256]   = W[g-slice, :].T with trailing bias row
  wo        bf16 [256, 1024]   = Wo[:, g-slice].T
  outT      f32  [1024, 2048]  = (Wo_g @ ctxn_g^T) partial, host transposes

Pipeline per core (PE never idles; DMA ordered to match consumption):
  K^T dch0,dch1 = Wk xk^T    (PE, psum accum; xk DMA'd first)
  Q^T dch0,dch1 = Wq xq^T
  per head pair (dch): per hf (q half):
    scores^T[k,q] = K_h^T-stationary @ Q_h^T   (64-row-tiled PE)
    attn = exp(scores/8)                        (ACT, psum->sbuf bf16)
    ctx_aug^T[d+1,q] += V_aug^T-stationary @ attn (psum accum, LAG behind)
    V projection grains interleave into head 0 (natural [s,d] layout)
    finished ctxn regions' out-projection grains interleave 1/chunk
  normalize: cp copy, reciprocal_approx_fast, gpsimd bcast, DVE mult
  tail (h3,hf1): ctx split into two 512-wide psums so normalize/outproj
  of the first half overlap the second half's PV.
"""

import numpy as np
import ml_dtypes

from concourse import bacc, tile, mybir
from concourse.bass_utils import run_bass_kernel_spmd

BF16 = mybir.dt.bfloat16
F32 = mybir.dt.float32
FP16 = mybir.dt.float16

S = 2048      # sequence length
D = 1024      # model dim
DG = 256      # per-core head-group width (4 heads x 64)
DK = 64       # head dim
NH = 4        # heads per core
MT = 8        # model-dim contraction tiles (1024 / 128)
KC = 16       # k chunks of 128
LAG = 4       # PV trails scores by LAG chunks
N_CORES = 8


def _emit_init(nc, pools, dram):
    """Iteration-invariant prologue: weight DMAs, ones row, bias row."""
    persist, xp, wp, wop, attnp, zp, outp, ps, ctxps, smallps = pools
    xq, xk, xv, wq, wk, wv, bT, wo, outT0, outT1 = dram

    ones = persist.tile([1, S], BF16, tag="ones", name="ones")
    nc.vector.memset(ones[:], 1.0)
    bt = persist.tile([1, 3 * DG], BF16, tag="bt", name="bt")
    nc.sync.dma_start(bt[:], bT[:])
    onesw = persist.tile([1, NH], BF16, tag="onesw", name="onesw")
    nc.vector.memset(onesw[:], 1.0)

    wts = {}
    for key, wdr in (("k", wk), ("q", wq), ("v", wv)):
        wt = []
        for m in range(MT):
            t = wp.tile([128, DG], BF16, tag="w", name="w")
            nc.sync.dma_start(t[:], wdr[m * 128:(m + 1) * 128, :])
            wt.append(t)
        wts[key] = wt

    wot = []
    for dch in range(2):
        t = wop.tile([128, D], BF16, tag="wo", name="wo")
        nc.sync.dma_start(t[:], wo[dch * 128:(dch + 1) * 128, :])
        wot.append(t)
    return ones, bt, onesw, wts, wot


def _emit(nc, pools, dram, init):
    persist, xp, wp, wop, attnp, zp, outp, ps, ctxps, smallps = pools
    xq, xk, xv, wq, wk, wv, bT, wo, outT0, outT1 = dram
    ones, bt, onesw, wts, wot = init
    HS = S // 2  # 1024-wide half grains

    # per-iteration persistent tiles (double-buffered across iterations)
    qt = [persist.tile([128, S], BF16, tag=f"qt{i}", name=f"qt{i}", bufs=2)
          for i in range(2)]
    kt = [persist.tile([128, S], BF16, tag=f"kt{i}", name=f"kt{i}", bufs=2)
          for i in range(2)]
    ctxn = [persist.tile([128, S], BF16, tag=f"ctxn{i}", name=f"ctxn{i}",
                         bufs=2) for i in range(2)]
    vaug = persist.tile([128, KC, NH, DK + 1], BF16, tag="vaug", name="vaug",
                        bufs=2)

    # input DMAs go on the gpsimd SW-DGE queue: the Sync queue carries the
    # output DMAs, and at the iteration seam the next body's input issues
    # would otherwise serialize behind ~32 queued output issues (~10us)
    xts = {}
    for key, xdr in (("k", xk), ("q", xq), ("v", xv)):
        xt = [xp.tile([128, S], BF16, tag="x", name=f"x{key}") for _ in range(MT)]
        for hf in range(2):
            for m in range(MT):
                nc.gpsimd.dma_start(xt[m][:, hf * HS:(hf + 1) * HS],
                                    xdr[m * 128:(m + 1) * 128, hf * HS:(hf + 1) * HS])
        xts[key] = xt

    # ---------------- K^T then Q^T projections (both dch up front) ---------
    # 512-wide psum grains, sharing the "ps" tag with the attention scores
    # psums: under cross-iteration overlap the next iteration's projections
    # run concurrently with this iteration's attention, and PSUM only has 8
    # banks (2 here + 4 ctx + 2 small).
    for bofs, (key, outsb) in enumerate((("k", kt), ("q", qt))):
        wt, xt = wts[key], xts[key]
        for dch in range(2):
            for hf in range(2):
                for qc in range(2):
                    psum = ps.tile([128, 512], F32, tag="ps", name="ps")
                    for m in range(MT):
                        nc.tensor.matmul(
                            psum[:],
                            wt[m][:, dch * 128:(dch + 1) * 128],
                            xt[m][:, hf * HS + qc * 512:hf * HS + (qc + 1) * 512],
                            start=(m == 0), stop=False)
                    bo_c = (1 - bofs) * DG + dch * 128  # bt: [bq, bk, bv]
                    nc.tensor.matmul(
                        psum[:],
                        bt[:, bo_c:bo_c + 128],
                        ones[:, hf * HS + qc * 512:hf * HS + (qc + 1) * 512],
                        start=False, stop=True)
                    nc.vector.tensor_copy(
                        outsb[dch][:, hf * HS + qc * 512:hf * HS + (qc + 1) * 512],
                        psum[:])

    # ---------------- V projection grains (interleaved into head 0) --------
    def vproj_grain(sc):
        xt = xts["v"]
        vps = smallps.tile([128, NH * (DK + 1)], F32, tag="sm", name="vps")
        for m in range(MT):
            nc.tensor.matmul(
                vps[:, 0:NH * DK],
                xt[m][:, sc * 128:(sc + 1) * 128],
                wts["v"][m][:],
                start=(m == 0), stop=False)
        nc.tensor.matmul(
            vps[:, 0:NH * DK],
            ones[:, sc * 128:(sc + 1) * 128],
            bt[:, 2 * DG:3 * DG],
            start=False, stop=True)
        # the softmax-denominator ones column, via a K=1 matmul (writing it
        # here instead of a per-iteration memset keeps vaug fully produced
        # by the loop body, so double-buffering works across iterations)
        nc.tensor.matmul(
            vps[:, NH * DK:NH * (DK + 1)],
            ones[:, sc * 128:(sc + 1) * 128],
            onesw[:],
            start=True, stop=True, skip_group_check=True)
        nc.vector.tensor_copy(vaug[:, sc, :, 0:DK],
                              vps[:, 0:NH * DK].rearrange(
                                  "p (h d) -> p h d", h=NH))
        nc.vector.tensor_copy(vaug[:, sc, :, DK:DK + 1],
                              vps[:, NH * DK:NH * (DK + 1)].rearrange(
                                  "p (h d) -> p h d", h=NH))

    # ---------------- out-projection grains --------------------------------
    def outproj_grain(dch, oc, qp, outT):
        """A [128, 1024] out-projection pair: two 512-wide psum grains,
        evicted into one SBUF tile, shipped with a single DMA."""
        osb = outp.tile([128, HS], BF16, tag="out", name="out")
        for j in range(2):
            ops = smallps.tile([128, 512], F32, tag="sm", name="ops")
            nc.tensor.matmul(
                ops[:], wot[dch][:, oc * 128:(oc + 1) * 128],
                ctxn[dch][:, qp * HS + j * 512:qp * HS + (j + 1) * 512],
                start=True, stop=True)
            nc.vector.tensor_copy(osb[:, j * 512:(j + 1) * 512], ops[:])
        nc.sync.dma_start(
            outT[oc * 128:(oc + 1) * 128, qp * HS:(qp + 1) * HS], osb[:])

    def outproj_half(dch, oc, qp, j, outT):
        """512-wide out-projection grain (tail variant)."""
        osb = outp.tile([128, HS], BF16, tag="out", name="outh")
        ops = smallps.tile([128, 512], F32, tag="sm", name="ops")
        nc.tensor.matmul(
            ops[:], wot[dch][:, oc * 128:(oc + 1) * 128],
            ctxn[dch][:, qp * HS + j * 512:qp * HS + (j + 1) * 512],
            start=True, stop=True)
        nc.vector.tensor_copy(osb[:, 0:512], ops[:])
        nc.sync.dma_start(
            outT[oc * 128:(oc + 1) * 128,
                 qp * HS + j * 512:qp * HS + (j + 1) * 512], osb[:, 0:512])

    def normalize(ctx_ap, dch, po, col0, width, via_cp=True, sfx=""):
        """ctxn[dch][po:po+DK, col0:col0+width] = ctx[0:DK] / ctx[DK].

        The [1, width] reciprocal is hostile to the DVE (single partition,
        ~6 cycles/elem -> 6.5us that clogs the in-order DVE FIFO), so the Z
        row takes a DMA round-trip through a [128, width/128] staging tile
        where the reciprocal runs across all partitions in ~0.1us.
        """
        if via_cp:
            cp = zp.tile([DK + 1, width], F32, tag="cp" + sfx, name="cp")
            nc.vector.tensor_copy(cp[:], ctx_ap)
            src = cp
            zrow = cp[DK:DK + 1, :]
        else:
            src = ctx_ap
            zrow_t = zp.tile([1, width], F32, tag="zrow" + sfx, name="zrow")
            nc.vector.tensor_copy(zrow_t[:], ctx_ap[DK:DK + 1, :])
            zrow = zrow_t[:]
        w = width // 128
        zst = zp.tile([128, w], F32, tag="zst" + sfx, name="zst")
        nc.gpsimd.dma_start(zst[:], zrow)
        rst = zp.tile([128, w], F32, tag="rst" + sfx, name="rst")
        nc.vector.reciprocal(rst[:], zst[:])
        zr = zp.tile([1, width], F32, tag="zr" + sfx, name="zr")
        nc.gpsimd.dma_start(zr[:], rst[:])
        bc = zp.tile([DK, width], F32, tag="bc" + sfx, name="bc")
        nc.gpsimd.partition_broadcast(bc[:], zr[:])
        nc.vector.tensor_mul(ctxn[dch][po:po + DK, col0:col0 + width],
                             src[0:DK, :], bc[:])

    # ---------------- attention, fused head pairs ---------------------------
    # Heads of a pair (dch) live on PE row-tiles T0 (partitions 0-63) and T8
    # (64-127); interleaving their scores matmuls makes the two 64-row tiles
    # stream concurrently (~2x scores throughput). Scores psums are evicted
    # to fp16 SBUF staging by the DVE immediately (freeing PSUM), and exp
    # runs on 2048-wide staged tiles, amortizing the ACT engine's 352-cycle
    # per-instruction overhead.
    from collections import deque
    pending = deque()
    for p in range(2):
        heads = ((2 * p, 0), (2 * p + 1, 64))
        for hf in range(2):
            last = (p == 1 and hf == 1)
            ctxs = [ctxps.tile([DK + 1, HS], F32, tag=f"ctx{t}",
                               name=f"ctx{t}") for t in range(2)]
            stage = [None, None]
            atts = {}
            for cc in range(KC + LAG):
                if cc < KC:
                    c = cc
                    if p == 0 and hf == 0:
                        vproj_grain(c)
                    if c % 2 == 0:
                        stage[0] = attnp.tile([128, S], BF16, tag="stg0",
                                              name="stg0")
                        stage[1] = attnp.tile([128, S], BF16, tag="stg1",
                                              name="stg1")
                    for qc in range(2):
                        for t, (hh, po) in enumerate(heads):
                            sp = ps.tile([128, 512], F32, tag="ps", name="ps")
                            nc.tensor.matmul(
                                sp[:],
                                kt[p][po:po + DK, c * 128:(c + 1) * 128],
                                qt[p][po:po + DK,
                                      hf * HS + qc * 512:hf * HS + (qc + 1) * 512],
                                start=True, stop=True)
                            nc.vector.tensor_copy(
                                stage[t][:, (c % 2) * HS + qc * 512:
                                         (c % 2) * HS + (qc + 1) * 512],
                                sp[:])
                    if c % 2 == 1:
                        for t in range(2):
                            att = attnp.tile([128, S], BF16, tag=f"att{t}",
                                             name=f"att{t}")
                            nc.scalar.activation(
                                att[:], stage[t][:],
                                mybir.ActivationFunctionType.Exp, scale=0.125)
                            atts[(t, c // 2)] = att
                if cc >= LAG:
                    c = cc - LAG
                    for qc in range(2):
                        for t, (hh, po) in enumerate(heads):
                            att = atts[(t, c // 2)]
                            nc.tensor.matmul(
                                ctxs[t][:, qc * 512:(qc + 1) * 512],
                                vaug[:, c, hh, :],
                                att[:, (c % 2) * HS + qc * 512:
                                    (c % 2) * HS + (qc + 1) * 512],
                                start=(c == 0), stop=(c == KC - 1))
                    if c % 2 == 1:
                        atts.pop((0, c // 2))
                        atts.pop((1, c // 2))
                if pending and cc >= 4 and cc % 2 == 0:
                    outproj_grain(*pending.popleft())
            for t, (hh, po) in enumerate(heads):
                normalize(ctxs[t][:], p, po, hf * HS, HS, via_cp=False,
                          sfx="B" if t else "")
            pending.extend((p, oc, hf, outT0 if p == 0 else outT1)
                           for oc in range(8))
    for g in pending:
        outproj_grain(*g)


def build_nc(reps=1):
    nc = bacc.Bacc("TRN2", target_bir_lowering=False)
    dram = (
        nc.dram_tensor("xq", [D, S], BF16, kind="ExternalInput"),
        nc.dram_tensor("xk", [D, S], BF16, kind="ExternalInput"),
        nc.dram_tensor("xv", [D, S], BF16, kind="ExternalInput"),
        nc.dram_tensor("wq", [D, DG], BF16, kind="ExternalInput"),
        nc.dram_tensor("wk", [D, DG], BF16, kind="ExternalInput"),
        nc.dram_tensor("wv", [D, DG], BF16, kind="ExternalInput"),
        nc.dram_tensor("bT", [1, 3 * DG], BF16, kind="ExternalInput"),
        nc.dram_tensor("wo", [DG, D], BF16, kind="ExternalInput"),
        nc.dram_tensor("outT0", [D, S], BF16, kind="ExternalOutput"),
        nc.dram_tensor("outT1", [D, S], BF16, kind="ExternalOutput"),
    )

    with tile.TileContext(nc) as tc:
        with (
            tc.tile_pool(name="persist", bufs=1) as persist,
            tc.tile_pool(name="xp", bufs=14) as xp,
            tc.tile_pool(name="wp", bufs=26) as wp,
            tc.tile_pool(name="wop", bufs=2) as wop,
            tc.tile_pool(name="attnp", bufs=2) as attnp,
            tc.tile_pool(name="zp", bufs=1) as zp,
            tc.tile_pool(name="outp", bufs=4) as outp,
            tc.tile_pool(name="ps", bufs=2, space="PSUM") as ps,
            tc.tile_pool(name="ctxps", bufs=1, space="PSUM") as ctxps,
            tc.tile_pool(name="smallps", bufs=2, space="PSUM") as smallps,
        ):
            pools = (persist, xp, wp, wop, attnp, zp, outp, ps, ctxps, smallps)
            init = _emit_init(nc, pools, dram)
            if reps == 1:
                _emit(nc, pools, dram, init)
            elif reps % 2 == 0:
                # two bodies per hardware-loop iteration: the loop-boundary
                # pipeline bubble (~11us) is paid once per two iterations
                with tc.For_i(0, reps // 2, 1):
                    _emit(nc, pools, dram, init)
                    _emit(nc, pools, dram, init)
            else:
                with tc.For_i(0, reps, 1):
                    _emit(nc, pools, dram, init)
    nc.compile()
    return nc


def make_in_maps(query, key, value, Wq, bq, Wk, bk, Wv, bv, Wo, bo):
    bf = ml_dtypes.bfloat16
    query, key, value = (np.asarray(a, np.float32) for a in (query, key, value))
    Wq, bq, Wk, bk, Wv, bv, Wo, bo = (
        np.asarray(a, np.float32) for a in (Wq, bq, Wk, bk, Wv, bv, Wo, bo))
    in_maps = []
    for c in range(N_CORES):
        b, g = divmod(c, 4)
        sl = slice(g * DG, (g + 1) * DG)

        def xa(x):
            return np.ascontiguousarray(x[b].T).astype(bf)

        def wa(W):
            return np.ascontiguousarray(W[sl, :].T).astype(bf)

        in_maps.append({
            "xq": xa(query), "xk": xa(key), "xv": xa(value),
            "wq": wa(Wq), "wk": wa(Wk), "wv": wa(Wv),
            "bT": np.concatenate([bq[sl], bk[sl], bv[sl]])[None, :].astype(bf),
            "wo": np.ascontiguousarray(Wo[:, sl].T).astype(bf),
        })
    return in_maps


_NC_CACHE = {}


def kernel(query, key, value, Wq, bq, Wk, bk, Wv, bv, Wo, bo):
    in_maps = make_in_maps(query, key, value, Wq, bq, Wk, bk, Wv, bv, Wo, bo)
    if 1 not in _NC_CACHE:
        _NC_CACHE[1] = build_nc(1)
    nc = _NC_CACHE[1]
    res = run_bass_kernel_spmd(nc, in_maps, core_ids=list(range(N_CORES)))
    out = np.zeros((2, S, D), np.float32)
    for c in range(N_CORES):
        b = c // 4
        out[b] += np.asarray(res.results[c]["outT0"], np.float32).T
        out[b] += np.asarray(res.results[c]["outT1"], np.float32).T
    out += np.asarray(bo, np.float32)[None, None, :]
    return out


# revision 29
# speedup vs baseline: 1.4837x; 1.3122x over previous
"""Multi-head attention (B=2, S=2048, D=1024, H=16) on 8 TRN2 NeuronCores.

Sharding: core c -> (batch b = c//4, head-group g = c%4). Each core computes
the attention output restricted to its batch and its 4 heads (a 256-wide
slice of the model dim), including the row-parallel output projection
partial product. Host sums the 4 partials per batch and adds bo.

Device-side layouts (everything transposed so no on-device transposes are
needed):
  xq/xk/xv  bf16 [1025, 2048]  = x[b].T with a trailing ones row (bias trick)
  wq/wk/wv  bf16 [1025, 256]   = W[g-slice, :].T with trailing bias row
  wo        bf16 [256, 1024]   = Wo[:, g-slice].T
  outT      f32  [1024, 2048]  = (Wo_g @ ctxn_g^T) partial, host transposes

Pipeline per core (PE never idles; DMA ordered to match consumption):
  K^T dch0,dch1 = Wk xk^T    (PE, psum accum; xk DMA'd first)
  Q^T dch0,dch1 = Wq xq^T
  per head pair (dch): per hf (q half):
    scores^T[k,q] = K_h^T-stationary @ Q_h^T   (64-row-tiled PE)
    attn = exp(scores/8)                        (ACT, psum->sbuf bf16)
    ctx_aug^T[d+1,q] += V_aug^T-stationary @ attn (psum accum, LAG behind)
    V projection grains interleave into head 0 (natural [s,d] layout)
    finished ctxn regions' out-projection grains interleave 1/chunk
  normalize: cp copy, reciprocal_approx_fast, gpsimd bcast, DVE mult
  tail (h3,hf1): ctx split into two 512-wide psums so normalize/outproj
  of the first half overlap the second half's PV.
"""

import numpy as np
import ml_dtypes

from concourse import bacc, tile, mybir
from concourse.bass_utils import run_bass_kernel_spmd

BF16 = mybir.dt.bfloat16
F32 = mybir.dt.float32

S = 2048      # sequence length
D = 1024      # model dim
DG = 256      # per-core head-group width (4 heads x 64)
DK = 64       # head dim
NH = 4        # heads per core
MT = 8        # model-dim contraction tiles (1024 / 128)
KC = 16       # k chunks of 128
LAG = 4       # PV trails scores by LAG chunks
N_CORES = 8


def _emit_init(nc, pools, dram):
    """Iteration-invariant prologue: weight DMAs, ones row, bias row."""
    persist, xp, wp, wop, attnp, zp, outp, ps, ctxps, smallps = pools
    xq, xk, xv, wq, wk, wv, bT, wo, outT0, outT1 = dram

    ones = persist.tile([1, S], BF16, tag="ones", name="ones")
    nc.vector.memset(ones[:], 1.0)
    bt = persist.tile([1, 3 * DG], BF16, tag="bt", name="bt")
    nc.sync.dma_start(bt[:], bT[:])
    onesw = persist.tile([1, NH], BF16, tag="onesw", name="onesw")
    nc.vector.memset(onesw[:], 1.0)

    wts = {}
    for key, wdr in (("k", wk), ("q", wq), ("v", wv)):
        wt = []
        for m in range(MT):
            t = wp.tile([128, DG], BF16, tag="w", name="w")
            nc.sync.dma_start(t[:], wdr[m * 128:(m + 1) * 128, :])
            wt.append(t)
        wts[key] = wt

    wot = []
    for dch in range(2):
        t = wop.tile([128, D], BF16, tag="wo", name="wo")
        nc.sync.dma_start(t[:], wo[dch * 128:(dch + 1) * 128, :])
        wot.append(t)
    return ones, bt, onesw, wts, wot


def _emit(nc, pools, dram, init):
    persist, xp, wp, wop, attnp, zp, outp, ps, ctxps, smallps = pools
    xq, xk, xv, wq, wk, wv, bT, wo, outT0, outT1 = dram
    ones, bt, onesw, wts, wot = init
    HS = S // 2  # 1024-wide half grains

    # per-iteration persistent tiles (double-buffered across iterations)
    qt = [persist.tile([128, S], BF16, tag=f"qt{i}", name=f"qt{i}", bufs=2)
          for i in range(2)]
    kt = [persist.tile([128, S], BF16, tag=f"kt{i}", name=f"kt{i}", bufs=2)
          for i in range(2)]
    ctxn = [persist.tile([128, S], BF16, tag=f"ctxn{i}", name=f"ctxn{i}",
                         bufs=2) for i in range(2)]
    vaug = persist.tile([128, KC, NH, DK + 1], BF16, tag="vaug", name="vaug",
                        bufs=2)

    # input DMAs go on the gpsimd SW-DGE queue: the Sync queue carries the
    # output DMAs, and at the iteration seam the next body's input issues
    # would otherwise serialize behind ~32 queued output issues (~10us)
    xts = {}
    for key, xdr in (("k", xk), ("q", xq), ("v", xv)):
        xt = [xp.tile([128, S], BF16, tag="x", name=f"x{key}") for _ in range(MT)]
        for hf in range(2):
            for m in range(MT):
                nc.gpsimd.dma_start(xt[m][:, hf * HS:(hf + 1) * HS],
                                    xdr[m * 128:(m + 1) * 128, hf * HS:(hf + 1) * HS])
        xts[key] = xt

    # ---------------- K^T then Q^T projections (both dch up front) ---------
    for bofs, (key, outsb) in enumerate((("k", kt), ("q", qt))):
        wt, xt = wts[key], xts[key]
        for dch in range(2):
            for hf in range(2):
                psum = ps.tile([128, HS], F32, tag="ps", name="ps")
                for m in range(MT):
                    for qc in range(2):
                        nc.tensor.matmul(
                            psum[:, qc * 512:(qc + 1) * 512],
                            wt[m][:, dch * 128:(dch + 1) * 128],
                            xt[m][:, hf * HS + qc * 512:hf * HS + (qc + 1) * 512],
                            start=(m == 0), stop=False)
                bo_c = (1 - bofs) * DG + dch * 128  # bt: [bq, bk, bv]
                for qc in range(2):
                    nc.tensor.matmul(
                        psum[:, qc * 512:(qc + 1) * 512],
                        bt[:, bo_c:bo_c + 128],
                        ones[:, hf * HS + qc * 512:hf * HS + (qc + 1) * 512],
                        start=False, stop=True)
                nc.vector.tensor_copy(outsb[dch][:, hf * HS:(hf + 1) * HS],
                                      psum[:])

    # ---------------- V projection grains (interleaved into head 0) --------
    def vproj_grain(sc):
        xt = xts["v"]
        vps = smallps.tile([128, NH * (DK + 1)], F32, tag="sm", name="vps")
        for m in range(MT):
            nc.tensor.matmul(
                vps[:, 0:NH * DK],
                xt[m][:, sc * 128:(sc + 1) * 128],
                wts["v"][m][:],
                start=(m == 0), stop=False)
        nc.tensor.matmul(
            vps[:, 0:NH * DK],
            ones[:, sc * 128:(sc + 1) * 128],
            bt[:, 2 * DG:3 * DG],
            start=False, stop=True)
        # the softmax-denominator ones column, via a K=1 matmul (writing it
        # here instead of a per-iteration memset keeps vaug fully produced
        # by the loop body, so double-buffering works across iterations)
        nc.tensor.matmul(
            vps[:, NH * DK:NH * (DK + 1)],
            ones[:, sc * 128:(sc + 1) * 128],
            onesw[:],
            start=True, stop=True, skip_group_check=True)
        nc.vector.tensor_copy(vaug[:, sc, :, 0:DK],
                              vps[:, 0:NH * DK].rearrange(
                                  "p (h d) -> p h d", h=NH))
        nc.vector.tensor_copy(vaug[:, sc, :, DK:DK + 1],
                              vps[:, NH * DK:NH * (DK + 1)].rearrange(
                                  "p (h d) -> p h d", h=NH))

    # ---------------- out-projection grains --------------------------------
    def outproj_grain(dch, oc, qp, outT):
        """A [128, 1024] out-projection pair: two 512-wide psum grains,
        evicted into one SBUF tile, shipped with a single DMA."""
        osb = outp.tile([128, HS], BF16, tag="out", name="out")
        for j in range(2):
            ops = smallps.tile([128, 512], F32, tag="sm", name="ops")
            nc.tensor.matmul(
                ops[:], wot[dch][:, oc * 128:(oc + 1) * 128],
                ctxn[dch][:, qp * HS + j * 512:qp * HS + (j + 1) * 512],
                start=True, stop=True)
            nc.vector.tensor_copy(osb[:, j * 512:(j + 1) * 512], ops[:])
        nc.sync.dma_start(
            outT[oc * 128:(oc + 1) * 128, qp * HS:(qp + 1) * HS], osb[:])

    def outproj_half(dch, oc, qp, j, outT):
        """512-wide out-projection grain (tail variant)."""
        osb = outp.tile([128, HS], BF16, tag="out", name="outh")
        ops = smallps.tile([128, 512], F32, tag="sm", name="ops")
        nc.tensor.matmul(
            ops[:], wot[dch][:, oc * 128:(oc + 1) * 128],
            ctxn[dch][:, qp * HS + j * 512:qp * HS + (j + 1) * 512],
            start=True, stop=True)
        nc.vector.tensor_copy(osb[:, 0:512], ops[:])
        nc.sync.dma_start(
            outT[oc * 128:(oc + 1) * 128,
                 qp * HS + j * 512:qp * HS + (j + 1) * 512], osb[:, 0:512])

    def normalize(ctx_ap, dch, po, col0, width, via_cp=True, sfx=""):
        """ctxn[dch][po:po+DK, col0:col0+width] = ctx[0:DK] / ctx[DK].

        The [1, width] reciprocal is hostile to the DVE (single partition,
        ~6 cycles/elem -> 6.5us that clogs the in-order DVE FIFO), so the Z
        row takes a DMA round-trip through a [128, width/128] staging tile
        where the reciprocal runs across all partitions in ~0.1us.
        """
        if via_cp:
            cp = zp.tile([DK + 1, width], F32, tag="cp" + sfx, name="cp")
            nc.vector.tensor_copy(cp[:], ctx_ap)
            src = cp
            zrow = cp[DK:DK + 1, :]
        else:
            src = ctx_ap
            zrow_t = zp.tile([1, width], F32, tag="zrow" + sfx, name="zrow")
            nc.vector.tensor_copy(zrow_t[:], ctx_ap[DK:DK + 1, :])
            zrow = zrow_t[:]
        w = width // 128
        zst = zp.tile([128, w], F32, tag="zst" + sfx, name="zst")
        nc.gpsimd.dma_start(zst[:], zrow)
        rst = zp.tile([128, w], F32, tag="rst" + sfx, name="rst")
        nc.vector.reciprocal(rst[:], zst[:])
        zr = zp.tile([1, width], F32, tag="zr" + sfx, name="zr")
        nc.gpsimd.dma_start(zr[:], rst[:])
        bc = zp.tile([DK, width], F32, tag="bc" + sfx, name="bc")
        nc.gpsimd.partition_broadcast(bc[:], zr[:])
        nc.vector.tensor_mul(ctxn[dch][po:po + DK, col0:col0 + width],
                             src[0:DK, :], bc[:])

    # ---------------- attention per head ------------------------------------
    from collections import deque
    pending = deque()
    for h in range(NH):
        dch, po = h // 2, 64 * (h % 2)
        for hf in range(2):          # q-pass split: ctx only [65, 1024] psum
            last = (h == 3 and hf == 1)
            ctx = ctxps.tile([DK + 1, HS], F32, tag="ctx", name="ctx")
            atts = {}
            for cc in range(KC + LAG):
                if cc < KC:
                    c = cc
                    if h == 0 and hf == 0:
                        vproj_grain(c)
                    scs = ps.tile([128, HS], F32, tag="ps", name="ps")
                    for qc in range(2):
                        nc.tensor.matmul(
                            scs[:, qc * 512:(qc + 1) * 512],
                            kt[dch][po:po + DK, c * 128:(c + 1) * 128],
                            qt[dch][po:po + DK,
                                    hf * HS + qc * 512:hf * HS + (qc + 1) * 512],
                            start=True, stop=True)
                    att = attnp.tile([128, HS], BF16, tag="attn", name="attn")
                    nc.scalar.activation(att[:], scs[:],
                                         mybir.ActivationFunctionType.Exp,
                                         scale=0.125)
                    atts[c] = att
                if cc >= LAG:
                    c = cc - LAG
                    att = atts.pop(c)
                    for qc in range(2):
                        nc.tensor.matmul(
                            ctx[:, qc * 512:(qc + 1) * 512],
                            vaug[:, c, h, :],
                            att[:, qc * 512:(qc + 1) * 512],
                            start=(c == 0), stop=(c == KC - 1))
                # drain one grain per chunk, but not in the first chunks of a
                # block: the source ctxn region's normalize chain (cp copy,
                # reciprocal, broadcast, multiply ~10us) is still in flight
                # and an early grain would stall the in-order PE queue.
                if pending and cc >= 8:
                    outproj_grain(*pending.popleft())
            if last:
                # both 512-wide normalize chains issued back-to-back so they
                # pipeline through DVE/gpsimd while the PE runs qA's
                # out-projection grains; qB's grains follow immediately
                normalize(ctx[:, 0:512], dch, po, hf * HS, 512, via_cp=False)
                normalize(ctx[:, 512:HS], dch, po, hf * HS + 512, 512,
                          via_cp=False, sfx="B")
                for oc in range(8):
                    outproj_half(1, oc, 1, 0, outT1)
                for oc in range(8):
                    outproj_half(1, oc, 1, 1, outT1)
            else:
                normalize(ctx[:], dch, po, hf * HS, HS)
            if h == 1 and hf == 1:
                # ctxn[0] complete -> its 16 pairs can go
                pending.extend((0, oc, qp, outT0)
                               for qp in range(2) for oc in range(8))
            if h == 3 and hf == 0:
                # ctxn[1][:, 0:HS] complete -> its 8 pairs can go
                pending.extend((1, oc, 0, outT1) for oc in range(8))
    for g in pending:
        outproj_grain(*g)


def build_nc(reps=1):
    nc = bacc.Bacc("TRN2", target_bir_lowering=False)
    dram = (
        nc.dram_tensor("xq", [D, S], BF16, kind="ExternalInput"),
        nc.dram_tensor("xk", [D, S], BF16, kind="ExternalInput"),
        nc.dram_tensor("xv", [D, S], BF16, kind="ExternalInput"),
        nc.dram_tensor("wq", [D, DG], BF16, kind="ExternalInput"),
        nc.dram_tensor("wk", [D, DG], BF16, kind="ExternalInput"),
        nc.dram_tensor("wv", [D, DG], BF16, kind="ExternalInput"),
        nc.dram_tensor("bT", [1, 3 * DG], BF16, kind="ExternalInput"),
        nc.dram_tensor("wo", [DG, D], BF16, kind="ExternalInput"),
        nc.dram_tensor("outT0", [D, S], BF16, kind="ExternalOutput"),
        nc.dram_tensor("outT1", [D, S], BF16, kind="ExternalOutput"),
    )

    with tile.TileContext(nc) as tc:
        with (
            tc.tile_pool(name="persist", bufs=1) as persist,
            tc.tile_pool(name="xp", bufs=18) as xp,
            tc.tile_pool(name="wp", bufs=26) as wp,
            tc.tile_pool(name="wop", bufs=2) as wop,
            tc.tile_pool(name="attnp", bufs=9) as attnp,
            tc.tile_pool(name="zp", bufs=1) as zp,
            tc.tile_pool(name="outp", bufs=5) as outp,
            tc.tile_pool(name="ps", bufs=2, space="PSUM") as ps,
            tc.tile_pool(name="ctxps", bufs=1, space="PSUM") as ctxps,
            tc.tile_pool(name="smallps", bufs=2, space="PSUM") as smallps,
        ):
            pools = (persist, xp, wp, wop, attnp, zp, outp, ps, ctxps, smallps)
            init = _emit_init(nc, pools, dram)
            if reps == 1:
                _emit(nc, pools, dram, init)
            elif reps % 2 == 0:
                # two bodies per hardware-loop iteration: the loop-boundary
                # pipeline bubble (~11us) is paid once per two iterations
                with tc.For_i(0, reps // 2, 1):
                    _emit(nc, pools, dram, init)
                    _emit(nc, pools, dram, init)
            else:
                with tc.For_i(0, reps, 1):
                    _emit(nc, pools, dram, init)
    nc.compile()
    return nc


def make_in_maps(query, key, value, Wq, bq, Wk, bk, Wv, bv, Wo, bo):
    bf = ml_dtypes.bfloat16
    query, key, value = (np.asarray(a, np.float32) for a in (query, key, value))
    Wq, bq, Wk, bk, Wv, bv, Wo, bo = (
        np.asarray(a, np.float32) for a in (Wq, bq, Wk, bk, Wv, bv, Wo, bo))
    in_maps = []
    for c in range(N_CORES):
        b, g = divmod(c, 4)
        sl = slice(g * DG, (g + 1) * DG)

        def xa(x):
            return np.ascontiguousarray(x[b].T).astype(bf)

        def wa(W):
            return np.ascontiguousarray(W[sl, :].T).astype(bf)

        in_maps.append({
            "xq": xa(query), "xk": xa(key), "xv": xa(value),
            "wq": wa(Wq), "wk": wa(Wk), "wv": wa(Wv),
            "bT": np.concatenate([bq[sl], bk[sl], bv[sl]])[None, :].astype(bf),
            "wo": np.ascontiguousarray(Wo[:, sl].T).astype(bf),
        })
    return in_maps


_NC_CACHE = {}


def kernel(query, key, value, Wq, bq, Wk, bk, Wv, bv, Wo, bo):
    in_maps = make_in_maps(query, key, value, Wq, bq, Wk, bk, Wv, bv, Wo, bo)
    if 1 not in _NC_CACHE:
        _NC_CACHE[1] = build_nc(1)
    nc = _NC_CACHE[1]
    res = run_bass_kernel_spmd(nc, in_maps, core_ids=list(range(N_CORES)))
    out = np.zeros((2, S, D), np.float32)
    for c in range(N_CORES):
        b = c // 4
        out[b] += np.asarray(res.results[c]["outT0"], np.float32).T
        out[b] += np.asarray(res.results[c]["outT1"], np.float32).T
    out += np.asarray(bo, np.float32)[None, None, :]
    return out
